# revision 31
# baseline (speedup 1.0000x reference)
"""Trainium2 Bass kernel for nn_MoEBlock_30502857736769 (moe_routing).

Math (reference):
    out = sum_k v_k * relu(h @ wi^T + (h @ A_k^T) @ B_k^T) @ wo^T

Key algebraic restructuring (exact, since v_k >= 0 and wo is linear):
    wi'    = wi + B0 @ A0                  (expert-0 LoRA folded on the HOST)
    p      = h @ wi'^T                     (computed ONCE, shared by both experts)
    tdiff  = h @ [A1; A0]^T                (rank-32 LoRA projections, one matmul)
    act    = relu(v0*p) + relu(v1*(p + dl)),  dl = [B1, -B0] @ tdiff  (one PSUM matmul)
    out    = act @ wo^T                    (applied ONCE to the weighted sum)

This halves the dominant matmul FLOPs vs. the reference (which runs the full
FFN per expert), and the host-side fold removes the per-f-tile expert-0 LoRA
matmul pass (~5% of PE time). Sharding: pure data-parallel over the 16384
tokens across the 8 NeuronCores (weights replicated); no collectives needed.

Token chunking: 512-token chunk pairs. Stage 1 (and the LoRA passes) run at
N=512, halving the per-instruction LDWEIGHTS leak vs N=256. PSUM cannot hold
512 tokens x 1024 d of stage-2 accumulators, so stage 2 is split: sub-chunk A
(tokens 0:256) accumulates in dedicated PSUM during the f-loop as usual;
sub-chunk B's act tiles persist in SBUF and a dense second f-pass ("pass B")
computes its stage 2 afterwards, accumulating into the same 4-buffer PSUM
ring that serves the LoRA projection and the stage-1 tiles (8 banks total,
exactly the PSUM budget).

All layouts are pre-transposed/pre-tiled on the host so every DMA moves
>=2KB-contiguous runs per partition. Matmuls run in fp16 (full PE rate;
fp32 is 4x slower, fp8 DoubleRow is only 2x and fails the error budget
without compensation passes that erase the win), accumulating in fp32 PSUM.
"""

import numpy as np

# Problem constants (hardcoded per harness contract - no spec.json reads).
D_MODEL = 1024
D_FF = 4096
N_CORES = 8
B, S = 8, 2048
TOKENS = B * S            # 16384
T = TOKENS // N_CORES     # 2048 tokens per core

P = 128                   # SBUF/PE partition count
TC = 512                  # tokens per chunk pair


def build_program(v0: float, v1: float, t_per_core: int = T, tc: int = TC):
    """Build + compile the SPMD single-core Bass program.

    DRAM parameter layouts (all fp16 except the fp32 output):
      xR  [P, NCH, KD, tc]  hidden shard: xR[p,ch,kd,t] = x[ch*tc+t, kd*P+p]
      wiR [P, 16, KD, 256]  (wi + B_i0 @ A_i0)^T in 16 f-slabs
      woT [F, D]            wo^T (rows are 2KB-contiguous)
      aR  [P, KD, 128]      [A_i1; A_i0; 0...]^T pre-tiled over kd
      bTb [128, F]          [B_i1^T; -B_i0^T; 0...]  (adds l1-l0, tq rows 0:32)
    aR/bTb are zero-padded to M=128/K=128 so every matmul has a
    full-row-extent LDWEIGHTS (partial loads conflict with in-flight
    full-row matmuls and serialize at ~2x spacing - measured on HW).
      out [Tc, D]   fp32 output shard (natural token-major layout)
    """
    import concourse.mybir as mybir
    import concourse.tile as tile
    from concourse import bacc
    from concourse.bass import ts, ds

    dt = mybir.dt
    AF = mybir.ActivationFunctionType

    D, F = D_MODEL, D_FF
    KD = D // P            # 8 contraction tiles over d_model
    KF = F // P            # 32 tiles over d_ff
    NCH = t_per_core // tc # chunk pairs
    HA = tc // 2           # sub-chunk size (256)
    TT = HA // P           # 128-token tiles per sub-chunk (2)
    MD = dt.float16

    assert t_per_core % tc == 0 and tc == 2 * HA and HA % P == 0

    nc = bacc.Bacc("TRN2", target_bir_lowering=False, debug=False)

    xR = nc.dram_tensor("xR", [P, NCH, KD, tc], MD, kind="ExternalInput")
    wiR = nc.dram_tensor("wiR", [P, 16, KD, F // 16], MD, kind="ExternalInput")
    woT = nc.dram_tensor("woT", [F, D], MD, kind="ExternalInput")
    aR = nc.dram_tensor("aR", [P, KD, P], MD, kind="ExternalInput")
    bTb = nc.dram_tensor("bTb", [P, F], MD, kind="ExternalInput")
    out = nc.dram_tensor("out", [t_per_core, D], dt.float32, kind="ExternalOutput")
    AOT = mybir.AluOpType

    with tile.TileContext(nc) as tc_ctx:
        with (
            tc_ctx.tile_pool(name="wi", bufs=1) as wi_pool,
            tc_ctx.tile_pool(name="wo", bufs=1) as wo_pool,
            tc_ctx.tile_pool(name="lora_w", bufs=1) as lw_pool,
            tc_ctx.tile_pool(name="x", bufs=2) as x_pool,
            tc_ctx.tile_pool(name="tcat", bufs=2) as tq_pool,
            tc_ctx.tile_pool(name="actA", bufs=6) as actA_pool,
            tc_ctx.tile_pool(name="actB", bufs=34) as actB_pool,
            tc_ctx.tile_pool(name="a1", bufs=3) as a1_pool,
            tc_ctx.tile_pool(name="osb", bufs=4) as osb_pool,
            tc_ctx.tile_pool(name="ring", bufs=4, space="PSUM") as ring_pool,
            tc_ctx.tile_pool(name="ps2", bufs=2, space="PSUM") as ps2_pool,
        ):
            # ---- DMA order: everything chunk 0 needs first, then the bulk
            #      weights (16 MB). NOTE: each dma_start costs ~0.5us of
            #      serial sync-engine dispatch at the head, so head-critical
            #      transfers use FEW dma_starts.
            #
            # PE warm-up: the Tensor engine ramps 0.65 -> 1.2 -> 2.4 GHz
            # over ~3us of CONTINUOUS execution, and any idle gap resets the
            # ramp; real head matmuls otherwise pay ~2.8us of ramp tax while
            # ALSO idling for DMA. These dependency-free matmuls on a zeroed
            # scratch tile run from t~0.3us and finish the ramp inside the
            # DMA-wait window (batch 2 below bridges the x0-bulk wait).
            warm = lw_pool.tile([P, 640], MD)
            nc.gpsimd.memset(warm[:, :], 0.0)
            # Sizing: engine init delays the first PE instruction to ~7us;
            # aR + x0[0:2] are DMA-ready at ~10us, exactly when 7 ramping
            # matmuls (605 + 6x427ns) finish the 3us continuous-busy ramp.
            wp1 = ring_pool.tile([P, 512], dt.float32, tag="ring", name="wp1")
            NW1 = 7
            for i in range(NW1):
                nc.tensor.matmul(
                    wp1[:, :], warm[:, 0:P], warm[:, P:640],
                    start=(i == 0), stop=(i == NW1 - 1),
                )
            a_t = lw_pool.tile([P, KD, P], MD)
            nc.sync.dma_start(a_t[:, :, :], aR[:, :, :])
            # wi as 16 f-slabs of [P, KD, 256]; f-tile fi lives in slab
            # fi>>1 at column offset (fi&1)*128
            FS = F // 16
            wi_t = wi_pool.tile([P, 16, KD, FS], MD)
            wo_t = wo_pool.tile([P, KF, D], MD)    # woT as KF tiles of [128, D]
            x0_t = x_pool.tile([P, KD, tc], MD, tag="x", name="x_t")
            nc.sync.dma_start(x0_t[:, 0:2, :], xR[:, 0, 0:2, :])
            # slab 0 before the x0 bulk: f-tile 0/1's first stage-1 matmuls
            # (kd 0:2) then overlap the x0[2:8] stream
            nc.sync.dma_start(wi_t[:, 0, :, :], wiR[:, 0, :, :])
            nc.sync.dma_start(x0_t[:, 2:KD, :], xR[:, 0, 2:KD, :])

            def wi_eighth(j):
                nc.sync.dma_start(
                    wi_t[:, 2 * j:2 * j + 2, :, :], wiR[:, 2 * j:2 * j + 2, :, :]
                )

            def wo_tile(kf):
                nc.sync.dma_start(wo_t[:, kf, :], woT[ts(kf, P), :])

            nc.sync.dma_start(wi_t[:, 1, :, :], wiR[:, 1, :, :])
            bTb_t = lw_pool.tile([P, F], MD)
            nc.sync.dma_start(bTb_t[:, :], bTb[:, :])
            wi_eighth(1)
            next_wo = 0
            for j in range(2, 8):
                for _ in range(3):
                    wo_tile(next_wo); next_wo += 1
                wi_eighth(j)
            while next_wo < KF:
                wo_tile(next_wo); next_wo += 1

            # Chunk-pair prologue: x load + LoRA A projections + tq copy.
            # For chunk 0 the A-proj is emitted as a split PSUM group (kd 0:2
            # first, kd 2:8 later) so the PE starts as soon as the first x0
            # piece lands instead of waiting for the whole 2MB chunk.
            def s1_partial(p1, fi, x_t, kds, first):
                for kd in kds:
                    nc.tensor.matmul(
                        p1[:, :],
                        wi_t[:, fi >> 1, kd, ds((fi & 1) * P, P)],
                        x_t[:, kd, :],
                        start=(first and kd == kds[0]), stop=(kd == KD - 1),
                    )

            def chunk_prologue(ch):
                pre = {}
                if ch == 0:
                    # Split PSUM groups: emit everything that only needs
                    # aR + x0[0:2] + slab0 first, so the PE works while the
                    # x0 bulk (1.5MB) is still streaming in.
                    x_t = x0_t
                    pl = ring_pool.tile(
                        [P, tc], dt.float32, tag="ring", name="pl"
                    )
                    for kd in range(2):
                        nc.tensor.matmul(
                            pl[:, :], a_t[:, kd, :], x_t[:, kd, :],
                            start=(kd == 0), stop=False,
                        )
                    for fi in range(2):
                        p1 = ring_pool.tile(
                            [P, tc], dt.float32, tag="ring", name="p1"
                        )
                        s1_partial(p1, fi, x_t, [0, 1], first=True)
                        pre[fi] = p1
                    # warm-up batch 2: keeps the PE clock ramped across the
                    # x0-bulk DMA wait (reuses wp1's ring buffer, never read)
                    wp2 = ring_pool.tile(
                        [P, 512], dt.float32, tag="ring", name="wp2"
                    )
                    for i in range(9):
                        nc.tensor.matmul(
                            wp2[:, :], warm[:, 0:P], warm[:, P:640],
                            start=(i == 0), stop=(i == 8),
                        )
                    for kd in range(2, KD):
                        nc.tensor.matmul(
                            pl[:, :], a_t[:, kd, :], x_t[:, kd, :],
                            start=False, stop=(kd == KD - 1),
                        )
                else:
                    x_t = x_pool.tile([P, KD, tc], MD, tag="x", name="x_t")
                    nc.sync.dma_start(x_t[:, :, :], xR[:, ch, :, :])
                    pl = ring_pool.tile(
                        [P, tc], dt.float32, tag="ring", name="pl"
                    )
                    for kd in range(KD):
                        nc.tensor.matmul(
                            pl[:, :], a_t[:, kd, :], x_t[:, kd, :],
                            start=(kd == 0), stop=(kd == KD - 1),
                        )
                tq = tq_pool.tile([P, tc], MD, tag="tcat", name="tq")
                nc.gpsimd.memset(tq[:, :], 0.0)
                nc.scalar.copy(tq[0:32, :], pl[0:32, :])
                return x_t, tq, pre

            for ch in range(NCH):
                x_t, tq, pre = chunk_prologue(ch)

                # ---- stage-2 accumulators for sub-chunk A ----
                ps2s = [
                    ps2_pool.tile([P, D], dt.float32, tag="ps2", name="ps2")
                    for _ in range(TT)
                ]

                # Two-deep software pipeline over f-tiles (as before, at
                # tc=512): iter i emits s1(i), relu0(i) for both halves,
                # stage-2-A for i-2, bdiff + relu1 (DVE) for i-1.
                def emit_s2a(act_prev, fi_prev, tts=None):
                    for tt in (range(TT) if tts is None else tts):
                        for dh in range(D // 512):
                            nc.tensor.matmul(
                                ps2s[tt][:, ts(dh, 512)],
                                act_prev[:, ts(tt, P)],
                                wo_t[:, fi_prev, ts(dh, 512)],
                                start=(fi_prev == 0), stop=(fi_prev == KF - 1),
                            )

                def emit_bdiff(st):
                    p1_, aA_, aB_, fi_ = st
                    nc.tensor.matmul(
                        p1_[:, :], bTb_t[:, ts(fi_, P)], tq[:, :],
                        start=False, stop=True, skip_group_check=True,
                    )
                    a1_t = a1_pool.tile([P, tc], MD, tag="a1", name="a1_t")
                    nc.vector.tensor_scalar(
                        a1_t[:, :], p1_[:, :], 0.0, float(v1),
                        AOT.max, AOT.mult,
                    )
                    nc.vector.tensor_add(aA_[:, :], aA_[:, :], a1_t[:, 0:HA])
                    nc.vector.tensor_add(aB_[:, :], aB_[:, :], a1_t[:, HA:tc])

                prev = None       # (p1, actA, actB, fi) of f-tile i-1
                s2q = []          # (actA, fi) awaiting stage-2-A emission
                actBs = []        # act tiles of sub-chunk B, for pass B
                for fi in range(KF):
                    # p0^T tile = wi'_fi @ x   (l0 already folded into wi')
                    if fi in pre:
                        p1 = pre[fi]
                        s1_partial(p1, fi, x_t, list(range(2, KD)), first=False)
                    else:
                        p1 = ring_pool.tile(
                            [P, tc], dt.float32, tag="ring", name="p1"
                        )
                        s1_partial(p1, fi, x_t, list(range(KD)), first=True)
                    # act = v0 * relu(p0), halves to separate A/B tiles. ACT
                    # folds the scale inside the relu (valid for v0 >= 0); a
                    # negative v0 routes through sign-safe DVE max+mult.
                    aA = actA_pool.tile([P, HA], MD, tag="actA", name="aA")
                    aB = actB_pool.tile([P, HA], MD, tag="actB", name="aB")
                    if v0 >= 0:
                        nc.scalar.activation(
                            aA[:, :], p1[:, 0:HA], AF.Relu,
                            bias=0.0, scale=float(v0),
                        )
                        nc.scalar.activation(
                            aB[:, :], p1[:, HA:tc], AF.Relu,
                            bias=0.0, scale=float(v0),
                        )
                    else:
                        nc.vector.tensor_scalar(
                            aA[:, :], p1[:, 0:HA], 0.0, float(v0),
                            AOT.max, AOT.mult,
                        )
                        nc.vector.tensor_scalar(
                            aB[:, :], p1[:, HA:tc], 0.0, float(v0),
                            AOT.max, AOT.mult,
                        )
                    actBs.append(aB)
                    if len(s2q) >= 2:
                        emit_s2a(*s2q.pop(0))
                    if prev is not None:
                        emit_bdiff(prev)
                        s2q.append((prev[1], prev[3]))
                    prev = (p1, aA, aB, fi)
                # drain the pipeline, tt-major: each ps2s[tt] accumulator
                # receives its stop-matmul as early as possible so its
                # evacuation overlaps the remaining drain + pass-B matmuls.
                emit_bdiff(prev)
                s2q.append((prev[1], prev[3]))
                for tt in range(TT):
                    for act_prev, fi_prev in s2q:
                        emit_s2a(act_prev, fi_prev, tts=[tt])

                # ---- evacuate sub-chunk A (overlaps pass B's matmuls) ----
                base = ch * tc

                def evac(psrc, row0, dh):
                    osb = osb_pool.tile(
                        [P, 512], dt.float32, tag="osb", name="osb"
                    )
                    if dh % 2 == 0:
                        nc.vector.tensor_copy(osb[:, :], psrc)
                    else:
                        nc.scalar.copy(osb[:, :], psrc)
                    for rh in range(2):
                        nc.sync.dma_start(
                            out[ds(row0 + rh * 64, 64), ds(dh * 512, 512)],
                            osb[ds(rh * 64, 64), :],
                        )

                for tt in range(TT):
                    for dh in range(2):
                        evac(ps2s[tt][:, ts(dh, 512)], base + tt * P, dh)

                # ---- pass B: dense stage-2 for sub-chunk B, quarter-major
                #      so each PSUM ring quarter finishes (and evacuates)
                #      as early as possible.
                for q in range(2 * TT):
                    ttb, dh = q >> 1, q & 1
                    pb = ring_pool.tile(
                        [P, 512], dt.float32, tag="ring", name="pb"
                    )
                    for fi in range(KF):
                        nc.tensor.matmul(
                            pb[:, :],
                            actBs[fi][:, ts(ttb, P)],
                            wo_t[:, fi, ts(dh, 512)],
                            start=(fi == 0), stop=(fi == KF - 1),
                        )
                    evac(pb[:, :], base + HA + ttb * P, dh)

    nc.compile()
    return nc


_PROGRAM_CACHE = {}


def _get_program(v0: float, v1: float):
    key = (float(v0), float(v1))
    if key not in _PROGRAM_CACHE:
        _PROGRAM_CACHE[key] = build_program(v0, v1)
    return _PROGRAM_CACHE[key]


def prep_inputs(hidden_states, wi_w, wo_w, lora_As, lora_Bs,
                top_k_indices, top_k_values, t_per_core: int = T):
    """Host-side shard + layout prep. Returns (in_maps, v0, v1)."""
    h = np.ascontiguousarray(np.asarray(hidden_states, dtype=np.float32))
    wi = np.asarray(wi_w, dtype=np.float32)
    wo = np.asarray(wo_w, dtype=np.float32)
    As = np.asarray(lora_As, dtype=np.float32)
    Bs = np.asarray(lora_Bs, dtype=np.float32)
    idx = np.asarray(top_k_indices).astype(np.int64)
    vals = np.asarray(top_k_values, dtype=np.float32)

    i0, i1 = int(idx[0]), int(idx[1])
    v0, v1 = float(vals[0]), float(vals[1])

    A0, A1 = As[i0], As[i1]                                      # [16, D]
    B0, B1 = Bs[i0], Bs[i1]                                      # [F, 16]
    KD = D_MODEL // 128
    # Fold expert-0's LoRA into wi on the host: wi' = wi + B0 @ A0.
    wi_eff = wi + B0.astype(np.float64) @ A0.astype(np.float64)
    wiT = wi_eff.T.astype(np.float16)                            # [D, F]
    # [P, 16, KD, F/16]: wiR[p,g,kd,fe] = wiT[kd*128+p, g*256+fe]
    wiR = np.ascontiguousarray(
        wiT.reshape(KD, 128, 16, D_FF // 16).transpose(1, 2, 0, 3)
    )
    woT = np.ascontiguousarray(wo.T).astype(np.float16)          # [F, D]
    aT = np.zeros((D_MODEL, 128), dtype=np.float16)              # [D, 128]
    aT[:, 0:16] = A1.T.astype(np.float16)
    aT[:, 16:32] = A0.T.astype(np.float16)
    # [P, KD, 128]: aR[p,kd,r] = aT[kd*128+p, r]
    aR = np.ascontiguousarray(aT.reshape(KD, 128, 128).transpose(1, 0, 2))
    bTb = np.zeros((128, D_FF), dtype=np.float16)
    bTb[0:16] = B1.T.astype(np.float16)
    bTb[16:32] = (-B0.T).astype(np.float16)

    tokens = h.reshape(TOKENS, D_MODEL)
    n_cores = TOKENS // t_per_core
    nch = t_per_core // TC
    in_maps = []
    for c in range(n_cores):
        shard = tokens[c * t_per_core:(c + 1) * t_per_core]
        # [P, NCH, KD, tc]: xR[p,ch,kd,t] = shard[ch*tc+t, kd*128+p]
        xf16 = shard.astype(np.float16)
        xRc = np.ascontiguousarray(
            xf16.reshape(nch, TC, KD, 128).transpose(3, 0, 2, 1)
        )
        in_maps.append({
            "xR": xRc, "wiR": wiR, "woT": woT,
            "aR": aR, "bTb": bTb,
        })
    return in_maps, v0, v1


# test.py can flip these to profile the run.
TRACE = False
TRACE_CORES = None
LAST_RESULT = None


def kernel(hidden_states, wi_w, wo_w, lora_As, lora_Bs,
           top_k_indices, top_k_values):
    global LAST_RESULT
    from concourse.bass_utils import run_bass_kernel_spmd

    in_maps, v0, v1 = prep_inputs(
        hidden_states, wi_w, wo_w, lora_As, lora_Bs,
        top_k_indices, top_k_values,
    )
    nc = _get_program(v0, v1)
    res = run_bass_kernel_spmd(
        nc, in_maps, list(range(N_CORES)),
        trace=TRACE, trace_cores=TRACE_CORES,
    )
    LAST_RESULT = res
    out = np.concatenate([r["out"] for r in res.results], axis=0)
    return out.reshape(B, S, D_MODEL).astype(np.float32, copy=False)


# revision 33
# speedup vs baseline: 1.0090x; 1.0090x over previous
"""Trainium2 Bass kernel for nn_MoEBlock_30502857736769 (moe_routing).

Math (reference):
    out = sum_k v_k * relu(h @ wi^T + (h @ A_k^T) @ B_k^T) @ wo^T

Key algebraic restructuring (exact, since v_k >= 0 and wo is linear):
    wi'    = wi + B0 @ A0                  (expert-0 LoRA folded on the HOST)
    p      = h @ wi'^T                     (computed ONCE, shared by both experts)
    tdiff  = h @ [A1; A0]^T                (rank-32 LoRA projections, one matmul)
    act    = relu(v0*p) + relu(v1*(p + dl)),  dl = [B1, -B0] @ tdiff  (one PSUM matmul)
    out    = act @ wo^T                    (applied ONCE to the weighted sum)

This halves the dominant matmul FLOPs vs. the reference (which runs the full
FFN per expert), and the host-side fold removes the per-f-tile expert-0 LoRA
matmul pass (~5% of PE time). Sharding: pure data-parallel over the 16384
tokens across the 8 NeuronCores (weights replicated); no collectives needed.

Token chunking: 512-token chunk pairs. Stage 1 (and the LoRA passes) run at
N=512, halving the per-instruction LDWEIGHTS leak vs N=256. PSUM cannot hold
512 tokens x 1024 d of stage-2 accumulators, so stage 2 is split: sub-chunk A
(tokens 0:256) accumulates in dedicated PSUM during the f-loop as usual;
sub-chunk B's act tiles persist in SBUF and a dense second f-pass ("pass B")
computes its stage 2 afterwards, accumulating into the same 4-buffer PSUM
ring that serves the LoRA projection and the stage-1 tiles (8 banks total,
exactly the PSUM budget).

All layouts are pre-transposed/pre-tiled on the host so every DMA moves
>=2KB-contiguous runs per partition. Matmuls run in fp16 (full PE rate;
fp32 is 4x slower, fp8 DoubleRow is only 2x and fails the error budget
without compensation passes that erase the win), accumulating in fp32 PSUM.
"""

import numpy as np

# Problem constants (hardcoded per harness contract - no spec.json reads).
D_MODEL = 1024
D_FF = 4096
N_CORES = 8
B, S = 8, 2048
TOKENS = B * S            # 16384
T = TOKENS // N_CORES     # 2048 tokens per core

P = 128                   # SBUF/PE partition count
TC = 512                  # tokens per chunk pair


def build_program(v0: float, v1: float, t_per_core: int = T, tc: int = TC):
    """Build + compile the SPMD single-core Bass program.

    DRAM parameter layouts (all fp16 except the fp32 output):
      xR  [P, NCH, KD, tc]  hidden shard: xR[p,ch,kd,t] = x[ch*tc+t, kd*P+p]
      wiR [P, 16, KD, 256]  (wi + B_i0 @ A_i0)^T in 16 f-slabs
      woT [F, D]            wo^T (rows are 2KB-contiguous)
      aR  [P, KD, 128]      [A_i1; A_i0; 0...]^T pre-tiled over kd
      bTb [128, F]          [B_i1^T; -B_i0^T; 0...]  (adds l1-l0, tq rows 0:32)
    aR/bTb are zero-padded to M=128/K=128 so every matmul has a
    full-row-extent LDWEIGHTS (partial loads conflict with in-flight
    full-row matmuls and serialize at ~2x spacing - measured on HW).
      out [Tc, D]   fp32 output shard (natural token-major layout)
    """
    import concourse.mybir as mybir
    import concourse.tile as tile
    from concourse import bacc
    from concourse.bass import ts, ds

    dt = mybir.dt
    AF = mybir.ActivationFunctionType

    D, F = D_MODEL, D_FF
    KD = D // P            # 8 contraction tiles over d_model
    KF = F // P            # 32 tiles over d_ff
    NCH = t_per_core // tc # chunk pairs
    HA = tc // 2           # sub-chunk size (256)
    TT = HA // P           # 128-token tiles per sub-chunk (2)
    MD = dt.float16

    assert t_per_core % tc == 0 and tc == 2 * HA and HA % P == 0

    nc = bacc.Bacc("TRN2", target_bir_lowering=False, debug=False)

    xR = nc.dram_tensor("xR", [P, NCH, KD, tc], MD, kind="ExternalInput")
    wiR = nc.dram_tensor("wiR", [P, 16, KD, F // 16], MD, kind="ExternalInput")
    woT = nc.dram_tensor("woT", [F, D], MD, kind="ExternalInput")
    aR = nc.dram_tensor("aR", [P, KD, P], MD, kind="ExternalInput")
    bTb = nc.dram_tensor("bTb", [P, F], MD, kind="ExternalInput")
    out = nc.dram_tensor("out", [t_per_core, D], dt.float32, kind="ExternalOutput")
    AOT = mybir.AluOpType

    with tile.TileContext(nc) as tc_ctx:
        with (
            tc_ctx.tile_pool(name="wi", bufs=1) as wi_pool,
            tc_ctx.tile_pool(name="wo", bufs=1) as wo_pool,
            tc_ctx.tile_pool(name="lora_w", bufs=1) as lw_pool,
            tc_ctx.tile_pool(name="x", bufs=2) as x_pool,
            tc_ctx.tile_pool(name="tcat", bufs=2) as tq_pool,
            tc_ctx.tile_pool(name="actA", bufs=6) as actA_pool,
            tc_ctx.tile_pool(name="actB", bufs=34) as actB_pool,
            tc_ctx.tile_pool(name="a1", bufs=3) as a1_pool,
            tc_ctx.tile_pool(name="osb", bufs=4) as osb_pool,
            tc_ctx.tile_pool(name="ring", bufs=4, space="PSUM") as ring_pool,
            tc_ctx.tile_pool(name="ps2", bufs=2, space="PSUM") as ps2_pool,
        ):
            # ---- DMA order: everything chunk 0 needs first, then the bulk
            #      weights (16 MB). NOTE: each dma_start costs ~0.5us of
            #      serial sync-engine dispatch at the head, so head-critical
            #      transfers use FEW dma_starts.
            #
            # PE warm-up: the Tensor engine ramps 0.65 -> 1.2 -> 2.4 GHz
            # over ~3us of CONTINUOUS execution, and any idle gap resets the
            # ramp; real head matmuls otherwise pay ~2.8us of ramp tax while
            # ALSO idling for DMA. These dependency-free matmuls on a zeroed
            # scratch tile run from t~0.3us and finish the ramp inside the
            # DMA-wait window (batch 2 below bridges the x0-bulk wait).
            warm = lw_pool.tile([P, 640], MD)
            nc.gpsimd.memset(warm[:, :], 0.0)
            # Sizing: engine init delays the first PE instruction to ~7us;
            # aR + x0[0:2] are DMA-ready at ~10.2us; 9 ramping matmuls end
            # ~10.9us so the real A-proj queues behind with no seam gap
            # (a gap here resets the clock ramp).
            wp1 = ring_pool.tile([P, 512], dt.float32, tag="ring", name="wp1")
            NW1 = 9
            for i in range(NW1):
                nc.tensor.matmul(
                    wp1[:, :], warm[:, 0:P], warm[:, P:640],
                    start=(i == 0), stop=(i == NW1 - 1),
                )
            a_t = lw_pool.tile([P, KD, P], MD)
            nc.sync.dma_start(a_t[:, :, :], aR[:, :, :])
            # wi as 16 f-slabs of [P, KD, 256]; f-tile fi lives in slab
            # fi>>1 at column offset (fi&1)*128
            FS = F // 16
            wi_t = wi_pool.tile([P, 16, KD, FS], MD)
            wo_t = wo_pool.tile([P, KF, D], MD)    # woT as KF tiles of [128, D]
            x0_t = x_pool.tile([P, KD, tc], MD, tag="x", name="x_t")
            nc.sync.dma_start(x0_t[:, 0:2, :], xR[:, 0, 0:2, :])
            # slab 0 before the x0 bulk: f-tile 0/1's first stage-1 matmuls
            # (kd 0:2) then overlap the x0[2:8] stream
            nc.sync.dma_start(wi_t[:, 0, :, :], wiR[:, 0, :, :])
            nc.sync.dma_start(x0_t[:, 2:KD, :], xR[:, 0, 2:KD, :])

            def wi_eighth(j):
                nc.sync.dma_start(
                    wi_t[:, 2 * j:2 * j + 2, :, :], wiR[:, 2 * j:2 * j + 2, :, :]
                )

            def wo_tile(kf):
                nc.sync.dma_start(wo_t[:, kf, :], woT[ts(kf, P), :])

            nc.sync.dma_start(wi_t[:, 1, :, :], wiR[:, 1, :, :])
            bTb_t = lw_pool.tile([P, F], MD)
            nc.sync.dma_start(bTb_t[:, :], bTb[:, :])
            wi_eighth(1)
            next_wo = 0
            for j in range(2, 8):
                for _ in range(3):
                    wo_tile(next_wo); next_wo += 1
                wi_eighth(j)
            while next_wo < KF:
                wo_tile(next_wo); next_wo += 1

            # Chunk-pair prologue: x load + LoRA A projections + tq copy.
            # For chunk 0 the A-proj is emitted as a split PSUM group (kd 0:2
            # first, kd 2:8 later) so the PE starts as soon as the first x0
            # piece lands instead of waiting for the whole 2MB chunk.
            def s1_partial(p1, fi, x_t, kds, first):
                for kd in kds:
                    nc.tensor.matmul(
                        p1[:, :],
                        wi_t[:, fi >> 1, kd, ds((fi & 1) * P, P)],
                        x_t[:, kd, :],
                        start=(first and kd == kds[0]), stop=(kd == KD - 1),
                    )

            def chunk_prologue(ch):
                pre = {}
                if ch == 0:
                    # Split PSUM groups: emit everything that only needs
                    # aR + x0[0:2] + slab0 first, so the PE works while the
                    # x0 bulk (1.5MB) is still streaming in.
                    x_t = x0_t
                    pl = ring_pool.tile(
                        [P, tc], dt.float32, tag="ring", name="pl"
                    )
                    for kd in range(2):
                        nc.tensor.matmul(
                            pl[:, :], a_t[:, kd, :], x_t[:, kd, :],
                            start=(kd == 0), stop=False,
                        )
                    # warm-up batch 2a bridges the A-proj -> slab0 DMA wait
                    # (~2us); 2b bridges the x0-bulk wait. Both write the
                    # same never-read scratch tile: a fresh ring slot here
                    # would WAW-block against the still-unconsumed pl.
                    wp2 = ring_pool.tile(
                        [P, 512], dt.float32, tag="ring", name="wp2"
                    )
                    for i in range(7):
                        nc.tensor.matmul(
                            wp2[:, :], warm[:, 0:P], warm[:, P:640],
                            start=(i == 0), stop=(i == 6),
                        )
                    for fi in range(2):
                        p1 = ring_pool.tile(
                            [P, tc], dt.float32, tag="ring", name="p1"
                        )
                        s1_partial(p1, fi, x_t, [0, 1], first=True)
                        pre[fi] = p1
                    for i in range(2):
                        nc.tensor.matmul(
                            wp2[:, :], warm[:, 0:P], warm[:, P:640],
                            start=(i == 0), stop=(i == 1),
                        )
                    for kd in range(2, KD):
                        nc.tensor.matmul(
                            pl[:, :], a_t[:, kd, :], x_t[:, kd, :],
                            start=False, stop=(kd == KD - 1),
                        )
                else:
                    x_t = x_pool.tile([P, KD, tc], MD, tag="x", name="x_t")
                    nc.sync.dma_start(x_t[:, :, :], xR[:, ch, :, :])
                    pl = ring_pool.tile(
                        [P, tc], dt.float32, tag="ring", name="pl"
                    )
                    for kd in range(KD):
                        nc.tensor.matmul(
                            pl[:, :], a_t[:, kd, :], x_t[:, kd, :],
                            start=(kd == 0), stop=(kd == KD - 1),
                        )
                tq = tq_pool.tile([P, tc], MD, tag="tcat", name="tq")
                nc.gpsimd.memset(tq[:, :], 0.0)
                nc.scalar.copy(tq[0:32, :], pl[0:32, :])
                return x_t, tq, pre

            for ch in range(NCH):
                x_t, tq, pre = chunk_prologue(ch)

                # ---- stage-2 accumulators for sub-chunk A ----
                ps2s = [
                    ps2_pool.tile([P, D], dt.float32, tag="ps2", name="ps2")
                    for _ in range(TT)
                ]

                # Two-deep software pipeline over f-tiles (as before, at
                # tc=512): iter i emits s1(i), relu0(i) for both halves,
                # stage-2-A for i-2, bdiff + relu1 (DVE) for i-1.
                def emit_s2a(act_prev, fi_prev, tts=None):
                    for tt in (range(TT) if tts is None else tts):
                        for dh in range(D // 512):
                            nc.tensor.matmul(
                                ps2s[tt][:, ts(dh, 512)],
                                act_prev[:, ts(tt, P)],
                                wo_t[:, fi_prev, ts(dh, 512)],
                                start=(fi_prev == 0), stop=(fi_prev == KF - 1),
                            )

                def emit_bdiff(st):
                    p1_, aA_, aB_, fi_ = st
                    nc.tensor.matmul(
                        p1_[:, :], bTb_t[:, ts(fi_, P)], tq[:, :],
                        start=False, stop=True, skip_group_check=True,
                    )
                    a1_t = a1_pool.tile([P, tc], MD, tag="a1", name="a1_t")
                    nc.vector.tensor_scalar(
                        a1_t[:, :], p1_[:, :], 0.0, float(v1),
                        AOT.max, AOT.mult,
                    )
                    nc.vector.tensor_add(aA_[:, :], aA_[:, :], a1_t[:, 0:HA])
                    nc.vector.tensor_add(aB_[:, :], aB_[:, :], a1_t[:, HA:tc])

                prev = None       # (p1, actA, actB, fi) of f-tile i-1
                s2q = []          # (actA, fi) awaiting stage-2-A emission
                actBs = []        # act tiles of sub-chunk B, for pass B
                for fi in range(KF):
                    # p0^T tile = wi'_fi @ x   (l0 already folded into wi')
                    if fi in pre:
                        p1 = pre[fi]
                        s1_partial(p1, fi, x_t, list(range(2, KD)), first=False)
                    else:
                        p1 = ring_pool.tile(
                            [P, tc], dt.float32, tag="ring", name="p1"
                        )
                        s1_partial(p1, fi, x_t, list(range(KD)), first=True)
                    # act = v0 * relu(p0), halves to separate A/B tiles. ACT
                    # folds the scale inside the relu (valid for v0 >= 0); a
                    # negative v0 routes through sign-safe DVE max+mult.
                    aA = actA_pool.tile([P, HA], MD, tag="actA", name="aA")
                    aB = actB_pool.tile([P, HA], MD, tag="actB", name="aB")
                    if v0 >= 0:
                        nc.scalar.activation(
                            aA[:, :], p1[:, 0:HA], AF.Relu,
                            bias=0.0, scale=float(v0),
                        )
                        nc.scalar.activation(
                            aB[:, :], p1[:, HA:tc], AF.Relu,
                            bias=0.0, scale=float(v0),
                        )
                    else:
                        nc.vector.tensor_scalar(
                            aA[:, :], p1[:, 0:HA], 0.0, float(v0),
                            AOT.max, AOT.mult,
                        )
                        nc.vector.tensor_scalar(
                            aB[:, :], p1[:, HA:tc], 0.0, float(v0),
                            AOT.max, AOT.mult,
                        )
                    actBs.append(aB)
                    if len(s2q) >= 2:
                        emit_s2a(*s2q.pop(0))
                    if prev is not None:
                        emit_bdiff(prev)
                        s2q.append((prev[1], prev[3]))
                    prev = (p1, aA, aB, fi)
                # drain the pipeline, tt-major: each ps2s[tt] accumulator
                # receives its stop-matmul as early as possible so its
                # evacuation overlaps the remaining drain + pass-B matmuls.
                emit_bdiff(prev)
                s2q.append((prev[1], prev[3]))
                for tt in range(TT):
                    for act_prev, fi_prev in s2q:
                        emit_s2a(act_prev, fi_prev, tts=[tt])

                # ---- evacuate sub-chunk A (overlaps pass B's matmuls) ----
                base = ch * tc

                def evac(psrc, row0, dh):
                    osb = osb_pool.tile(
                        [P, 512], dt.float32, tag="osb", name="osb"
                    )
                    if dh % 2 == 0:
                        nc.vector.tensor_copy(osb[:, :], psrc)
                    else:
                        nc.scalar.copy(osb[:, :], psrc)
                    for rh in range(2):
                        nc.sync.dma_start(
                            out[ds(row0 + rh * 64, 64), ds(dh * 512, 512)],
                            osb[ds(rh * 64, 64), :],
                        )

                for tt in range(TT):
                    for dh in range(2):
                        evac(ps2s[tt][:, ts(dh, 512)], base + tt * P, dh)

                # ---- pass B: dense stage-2 for sub-chunk B, quarter-major
                #      so each PSUM ring quarter finishes (and evacuates)
                #      as early as possible.
                for q in range(2 * TT):
                    ttb, dh = q >> 1, q & 1
                    pb = ring_pool.tile(
                        [P, 512], dt.float32, tag="ring", name="pb"
                    )
                    for fi in range(KF):
                        nc.tensor.matmul(
                            pb[:, :],
                            actBs[fi][:, ts(ttb, P)],
                            wo_t[:, fi, ts(dh, 512)],
                            start=(fi == 0), stop=(fi == KF - 1),
                        )
                    evac(pb[:, :], base + HA + ttb * P, dh)

    nc.compile()
    return nc


_PROGRAM_CACHE = {}


def _get_program(v0: float, v1: float):
    key = (float(v0), float(v1))
    if key not in _PROGRAM_CACHE:
        _PROGRAM_CACHE[key] = build_program(v0, v1)
    return _PROGRAM_CACHE[key]


def prep_inputs(hidden_states, wi_w, wo_w, lora_As, lora_Bs,
                top_k_indices, top_k_values, t_per_core: int = T):
    """Host-side shard + layout prep. Returns (in_maps, v0, v1)."""
    h = np.ascontiguousarray(np.asarray(hidden_states, dtype=np.float32))
    wi = np.asarray(wi_w, dtype=np.float32)
    wo = np.asarray(wo_w, dtype=np.float32)
    As = np.asarray(lora_As, dtype=np.float32)
    Bs = np.asarray(lora_Bs, dtype=np.float32)
    idx = np.asarray(top_k_indices).astype(np.int64)
    vals = np.asarray(top_k_values, dtype=np.float32)

    i0, i1 = int(idx[0]), int(idx[1])
    v0, v1 = float(vals[0]), float(vals[1])

    A0, A1 = As[i0], As[i1]                                      # [16, D]
    B0, B1 = Bs[i0], Bs[i1]                                      # [F, 16]
    KD = D_MODEL // 128
    # Fold expert-0's LoRA into wi on the host: wi' = wi + B0 @ A0.
    wi_eff = wi + B0.astype(np.float64) @ A0.astype(np.float64)
    wiT = wi_eff.T.astype(np.float16)                            # [D, F]
    # [P, 16, KD, F/16]: wiR[p,g,kd,fe] = wiT[kd*128+p, g*256+fe]
    wiR = np.ascontiguousarray(
        wiT.reshape(KD, 128, 16, D_FF // 16).transpose(1, 2, 0, 3)
    )
    woT = np.ascontiguousarray(wo.T).astype(np.float16)          # [F, D]
    aT = np.zeros((D_MODEL, 128), dtype=np.float16)              # [D, 128]
    aT[:, 0:16] = A1.T.astype(np.float16)
    aT[:, 16:32] = A0.T.astype(np.float16)
    # [P, KD, 128]: aR[p,kd,r] = aT[kd*128+p, r]
    aR = np.ascontiguousarray(aT.reshape(KD, 128, 128).transpose(1, 0, 2))
    bTb = np.zeros((128, D_FF), dtype=np.float16)
    bTb[0:16] = B1.T.astype(np.float16)
    bTb[16:32] = (-B0.T).astype(np.float16)

    tokens = h.reshape(TOKENS, D_MODEL)
    n_cores = TOKENS // t_per_core
    nch = t_per_core // TC
    in_maps = []
    for c in range(n_cores):
        shard = tokens[c * t_per_core:(c + 1) * t_per_core]
        # [P, NCH, KD, tc]: xR[p,ch,kd,t] = shard[ch*tc+t, kd*128+p]
        xf16 = shard.astype(np.float16)
        xRc = np.ascontiguousarray(
            xf16.reshape(nch, TC, KD, 128).transpose(3, 0, 2, 1)
        )
        in_maps.append({
            "xR": xRc, "wiR": wiR, "woT": woT,
            "aR": aR, "bTb": bTb,
        })
    return in_maps, v0, v1


# test.py can flip these to profile the run.
TRACE = False
TRACE_CORES = None
LAST_RESULT = None


def kernel(hidden_states, wi_w, wo_w, lora_As, lora_Bs,
           top_k_indices, top_k_values):
    global LAST_RESULT
    from concourse.bass_utils import run_bass_kernel_spmd

    in_maps, v0, v1 = prep_inputs(
        hidden_states, wi_w, wo_w, lora_As, lora_Bs,
        top_k_indices, top_k_values,
    )
    nc = _get_program(v0, v1)
    res = run_bass_kernel_spmd(
        nc, in_maps, list(range(N_CORES)),
        trace=TRACE, trace_cores=TRACE_CORES,
    )
    LAST_RESULT = res
    out = np.concatenate([r["out"] for r in res.results], axis=0)
    return out.reshape(B, S, D_MODEL).astype(np.float32, copy=False)
